# revision 1
# baseline (speedup 1.0000x reference)
"""Trainium2 Bass kernel for nn_MultiHeadCausalAttention (B=4, S=2048, D=1024, H=16).

Sharding: 8 cores = 4 (batch) x 2 (tensor-parallel over heads; 8 heads/core).
Per core:
  - QKV projections for its 8 heads, computed from x^T (host-transposed).
  - Flash-style causal attention in transposed-score layout: S^T = K @ Q^T with
    k on partitions, so exp(S^T) feeds the P^T @ V matmul directly (contraction
    over k) with no on-chip transposes.  Softmax row sums come for free from a
    ones-column appended to V; normalization is a rank-1 broadcast matmul of
    the reciprocal row times A^T.
  - AllGather (pairs) of the per-head attention outputs A^T so each core can
    apply the full out-projection for its 512 output columns (host-sliced Wo).
Raw Bass (no Tile): per-engine programs with hand-placed counting semaphores.
`reps` replays the body inside one NEFF (sem values offset per rep) so the
true per-iteration time can be measured as a slope, independent of the ~78 ms
axon dispatch floor.
Host: transposes x, slices/casts weights to bf16, assembles the output halves.
"""

from contextlib import ExitStack

import numpy as np
import ml_dtypes

import concourse.bass as bass
import concourse.mybir as mybir
from concourse.bass_utils import run_bass_kernel_spmd

F32 = mybir.dt.float32
F32R = mybir.dt.float32r
BF16 = mybir.dt.bfloat16
AF = mybir.ActivationFunctionType

B, S_FULL, D = 4, 2048, 1024
NCORES = 8
NDT = D // 128
DOWN = D // 2          # output dims owned per core (8 heads * 64)
SCALE = 1.0 / 32.0     # d_out ** -0.5
RG = [[0, 1], [2, 3], [4, 5], [6, 7]]


class Waiter:
    """Per-engine wait helper that elides waits already implied."""

    def __init__(self, eng):
        self.eng = eng
        self.seen = {}

    def __call__(self, sem, val):
        if val <= 0:
            return
        if self.seen.get(sem.name, -1) >= val:
            return
        self.seen[sem.name] = val
        self.eng.wait_ge(sem, val)


def build_program(S=S_FULL, reps=1):
    NQB = S // 512
    NST = S // 128
    NHP = 4
    NBLK = NHP * NQB

    # attention step list
    steps = []
    for hp in range(NHP):
        for qb in range(NQB):
            nkt = (qb + 1) * 4
            for kt in range(nkt):
                steps.append((hp, qb, kt, nkt, kt * 128 - qb * 512))
    NSTEPS = len(steps)
    diag_upto = []
    nd = 0
    for (hp, qb, kt, nkt, delta) in steps:
        if delta >= 0:
            nd += 1
        diag_upto.append(nd)
    ND = nd
    # last attention step index touching head-pair hp / V tile st
    hp_last = [max(i for i, s in enumerate(steps) if s[0] == hp) for hp in range(NHP)]
    vs_last = [
        max(i for i, s in enumerate(steps) if s[2] == st) for st in range(NST)
    ]

    # proj emission order shared by PE and DVE
    proj = []
    for hp in range(NHP):
        for sb in range(S // 512):
            proj.append(("q", hp, sb))
            proj.append(("k", hp, sb))
    for st in range(NST):
        proj.append(("v", st))
    NPJ = len(proj)

    nc = bass.Bass()
    xt = nc.declare_dram_parameter("xt", [D, S], BF16, isOutput=False)
    wq = nc.declare_dram_parameter("wq", [D, DOWN], BF16, isOutput=False)
    wk = nc.declare_dram_parameter("wk", [D, DOWN], BF16, isOutput=False)
    wv = nc.declare_dram_parameter("wv", [D, DOWN], BF16, isOutput=False)
    wo = nc.declare_dram_parameter("wo", [D, DOWN], BF16, isOutput=False)
    bob = nc.declare_dram_parameter("bob", [128, DOWN], F32, isOutput=False)
    tri = nc.declare_dram_parameter("tri", [128, 512], BF16, isOutput=False)
    ntri = nc.declare_dram_parameter("ntri", [128, 896], BF16, isOutput=False)
    negi = nc.declare_dram_parameter("negi", [128, 128], BF16, isOutput=False)
    one64 = nc.declare_dram_parameter("one64", [1, 64], F32R, isOutput=False)
    out = nc.declare_dram_parameter("out", [S, DOWN], F32, isOutput=True)

    cci = [nc.dram_tensor(f"cci{i}", [128, S], BF16) for i in range(NHP)]
    cco = [nc.dram_tensor(f"cco{i}", [256, S], BF16) for i in range(NHP)]

    with ExitStack() as ctx:
        e = ctx.enter_context
        ctx.enter_context(
            nc.allow_low_precision(reason="intentional bf16 flash attention")
        )

        sems = {}
        for n in (
            "dXQ", "dWK", "dWV", "dWO", "dMISC", "sPJ", "sPJC", "sPS", "sEX",
            "sMK", "sAV", "sRC", "sPB", "sA", "dCC", "sCG", "dAT", "sOP",
            "sOB", "dO0", "dO1",
        ):
            sems[n] = e(nc.semaphore(n))
        dXQ, dWK, dWV, dWO, dMISC = (sems[k] for k in ("dXQ", "dWK", "dWV", "dWO", "dMISC"))
        sPJ, sPJC, sPS, sEX, sMK = (sems[k] for k in ("sPJ", "sPJC", "sPS", "sEX", "sMK"))
        sAV, sRC, sPB, sA = (sems[k] for k in ("sAV", "sRC", "sPB", "sA"))
        dCC, sCG, dAT, sOP, sOB = (sems[k] for k in ("dCC", "sCG", "dAT", "sOP", "sOB"))
        dO = [sems["dO0"], sems["dO1"]]

        # one PSUM tensor, manual bank layout:
        # banks 0-3: proj psums (q/k) & attention score slots & outproj (0-1)
        # banks 4-7: proj psums (v), pav0, pav1, pb0, pb1 during attention
        P = e(nc.psum_tensor("P", [128, 4096], F32))

        def pav(r):
            return 2048 + 512 * r

        def pbc(r):
            return 3072 + 512 * r

        QT = [e(nc.sbuf_tensor(f"QT{i}", [128, S], BF16)) for i in range(NHP)]
        KT = [e(nc.sbuf_tensor(f"KT{i}", [128, S], BF16)) for i in range(NHP)]
        VS = [e(nc.sbuf_tensor(f"VS{i}", [128, 8 * 65], BF16)) for i in range(NST)]
        WOt = [e(nc.sbuf_tensor(f"WOt{i}", [128, DOWN], BF16)) for i in range(NDT)]
        tri_sb = e(nc.sbuf_tensor("tri_sb", [128, 512], BF16))
        ntri_sb = e(nc.sbuf_tensor("ntri_sb", [128, 896], BF16))
        negi_sb = e(nc.sbuf_tensor("negi_sb", [128, 128], BF16))
        one64_sb = e(nc.sbuf_tensor("one64_sb", [1, 64], F32R))
        bob_sb = e(nc.sbuf_tensor("bob_sb", [128, DOWN], F32))
        PT = [e(nc.sbuf_tensor(f"PT{i}", [128, 1024], BF16)) for i in range(6)]
        RCM = e(nc.sbuf_tensor("RCM", [1, 1024], F32R))
        PBS = [e(nc.sbuf_tensor(f"PBS{i}", [64, 512], F32)) for i in range(2)]
        OSB = [e(nc.sbuf_tensor(f"OSB{i}", [128, DOWN], F32)) for i in range(2)]
        XT = [e(nc.sbuf_tensor(f"XT{i}", [128, S], BF16)) for i in range(NDT)]
        WQt = [e(nc.sbuf_tensor(f"WQt{i}", [128, DOWN], BF16)) for i in range(NDT)]
        WKt = [e(nc.sbuf_tensor(f"WKt{i}", [128, DOWN], BF16)) for i in range(NDT)]
        WVt = [e(nc.sbuf_tensor(f"WVt{i}", [128, DOWN], BF16)) for i in range(NDT)]
        ATB = [e(nc.sbuf_tensor(f"ATB{i}", [128, S], BF16)) for i in range(NDT)]
        # A^T head pairs: head 2hp+r lives on partitions r*64:(r+1)*64
        ASBP = [e(nc.sbuf_tensor(f"ASBP{i}", [128, S], BF16)) for i in range(NHP)]

        with nc.Block() as blk:

            @blk.sync
            def _(sync):
                w = Waiter(sync)
                for r in range(reps):
                    if r == 0:
                        for i in range(NDT):
                            sl = slice(i * 128, (i + 1) * 128)
                            sync.dma_start(XT[i][:], xt[sl, :]).then_inc(dXQ, 16)
                            sync.dma_start(WQt[i][:], wq[sl, :]).then_inc(dXQ, 16)
                        for i in range(NDT):
                            sl = slice(i * 128, (i + 1) * 128)
                            sync.dma_start(WKt[i][:], wk[sl, :]).then_inc(dWK, 16)
                        for i in range(NDT):
                            sl = slice(i * 128, (i + 1) * 128)
                            sync.dma_start(WVt[i][:], wv[sl, :]).then_inc(dWV, 16)
                        sync.dma_start(tri_sb[:], tri[:]).then_inc(dMISC, 16)
                        sync.dma_start(ntri_sb[:], ntri[:]).then_inc(dMISC, 16)
                        sync.dma_start(negi_sb[:], negi[:]).then_inc(dMISC, 16)
                        sync.dma_start(one64_sb[:], one64[:]).then_inc(dMISC, 16)
                        sync.dma_start(bob_sb[:], bob[:]).then_inc(dMISC, 16)
                        for i in range(NDT):
                            sl = slice(i * 128, (i + 1) * 128)
                            sync.dma_start(WOt[i][:], wo[sl, :]).then_inc(dWO, 16)
                    for hp in range(NHP):
                        w(sA, 2 * (r * NBLK + hp * NQB + NQB))
                        if r > 0:
                            w(sCG, NHP * (r - 1) + hp + 1)  # cci free
                        sync.dma_start(cci[hp][:], ASBP[hp][:]).then_inc(dCC, 16)
                    if r + 1 < reps:
                        # re-stream x for the next rep (steady-state measurement)
                        w(sPJ, (r + 1) * NPJ)  # this rep's proj done reading XT
                        for i in range(NDT):
                            sl = slice(i * 128, (i + 1) * 128)
                            sync.dma_start(XT[i][:], xt[sl, :]).then_inc(dXQ, 16)
                    for qt in range(NST):
                        gq = r * NST + qt
                        w(sOB, gq + 1)
                        sync.dma_start(
                            out[qt * 128 : (qt + 1) * 128, :], OSB[qt % 2][:]
                        ).then_inc(dO[qt % 2], 16)
                w(dO[0], 16 * reps * ((NST + 1) // 2))
                w(dO[1], 16 * reps * (NST // 2))

            @blk.gpsimd
            def _(gpsimd):
                w = Waiter(gpsimd)
                for r in range(reps):
                    for hp in range(NHP):
                        w(dCC, 16 * (NHP * r + hp + 1))
                        if r > 0:
                            w(dAT, 32 * (NHP * (r - 1) + hp + 1))  # cco free
                        gpsimd.collective_compute(
                            "AllGather",
                            mybir.AluOpType.bypass,
                            replica_groups=RG,
                            ins=[cci[hp][:]],
                            outs=[cco[hp][:]],
                        ).then_inc(sCG, 1)
                        w(sCG, NHP * r + hp + 1)
                        if r > 0:
                            w(sOP, r * NST)  # ATB free (prev outproj done)
                        gpsimd.dma_start(ATB[hp][:], cco[hp][0:128, :]).then_inc(
                            dAT, 16
                        )
                        gpsimd.dma_start(
                            ATB[hp + 4][:], cco[hp][128:256, :]
                        ).then_inc(dAT, 16)

            @blk.tensor
            def _(tensor):
                w = Waiter(tensor)
                for r in range(reps):
                    # psum banks from previous rep fully consumed?
                    if r > 0:
                        w(sEX, r * NSTEPS)
                        w(sA, 2 * r * NBLK)
                        w(sOB, r * NST)
                    def dxq_val(dt):
                        # bulk wait: DMA queue completions are unordered, so
                        # only the all-issued count is a sound threshold
                        return 256 + 128 * r

                    for j, item in enumerate(proj):
                        gj = r * NPJ + j
                        bank = (j % 4) if item[0] != "v" else 4 + (j % 4)
                        pslc = slice(bank * 512, bank * 512 + 512)
                        w(sPJC, gj - 3)
                        if item[0] in ("q", "k"):
                            kind, hp, sb = item
                            wt = WQt if kind == "q" else WKt
                            hsl = slice(hp * 128, (hp + 1) * 128)
                            ssl = slice(sb * 512, (sb + 1) * 512)
                            for dt in range(NDT):
                                w(dXQ, dxq_val(dt))
                                if kind == "k":
                                    w(dWK, 128)
                                mm = nc.tensor.matmul(
                                    P[:, pslc],
                                    lhsT=wt[dt][:, hsl],
                                    rhs=XT[dt][:, ssl],
                                    start=(dt == 0),
                                    stop=(dt == NDT - 1),
                                    skip_group_check=True,
                                )
                            mm.then_inc(sPJ, 1)
                        else:
                            _, st = item
                            stsl = slice(st * 128, (st + 1) * 128)
                            for dt in range(NDT):
                                w(dXQ, dxq_val(dt))
                                w(dWV, 128)
                                mm = nc.tensor.matmul(
                                    P[:, pslc],
                                    lhsT=XT[dt][:, stsl],
                                    rhs=WVt[dt][:],
                                    start=(dt == 0),
                                    stop=(dt == NDT - 1),
                                    skip_group_check=True,
                                )
                            mm.then_inc(sPJ, 1)
                    # attention (PE software-pipelined: scores run one
                    # step ahead of the AV matmuls so ACT exp overlaps PE)
                    w(sPJC, (r + 1) * NPJ)
                    w(dMISC, 80)

                    def emit_scores(i):
                        hp, qb, kt, nkt, delta = steps[i]
                        gi = r * NSTEPS + i
                        s = i % 2
                        qsl = slice(qb * 512, (qb + 1) * 512)
                        ksl = slice(kt * 128, (kt + 1) * 128)
                        w(sEX, gi - 1)
                        diag = delta >= 0
                        for rr in range(2):
                            psl = slice(rr * 64, (rr + 1) * 64)
                            mm = nc.tensor.matmul(
                                P[:, s * 1024 + rr * 512 : s * 1024 + rr * 512 + 512],
                                lhsT=KT[hp][psl, ksl],
                                rhs=QT[hp][psl, qsl],
                                start=True,
                                stop=not diag,
                                tile_position=(rr * 64, 0),
                                skip_group_check=True,
                            )
                        if diag:
                            # causal mask: accumulate -BIG onto j < delta + p
                            wsl = slice(384, 896 - delta)
                            for rr in range(2):
                                base = s * 1024 + rr * 512
                                mm = nc.tensor.matmul(
                                    P[:, base + delta : base + 512],
                                    lhsT=negi_sb[:],
                                    rhs=ntri_sb[:, wsl],
                                    start=False,
                                    stop=True,
                                    skip_group_check=True,
                                )
                        mm.then_inc(sPS, 1)

                    emit_scores(0)
                    for i, (hp, qb, kt, nkt, delta) in enumerate(steps):
                        gi = r * NSTEPS + i
                        gblk = r * NBLK + hp * NQB + qb
                        w0 = max(delta, 0)
                        if i + 1 < NSTEPS:
                            emit_scores(i + 1)
                        w(sEX, gi + 1)
                        for rr in range(2):
                            h = 2 * hp + rr
                            w(sA, 2 * gblk - 1 + rr)
                            mm = nc.tensor.matmul(
                                P[0:65, pav(rr) + w0 : pav(rr) + 512],
                                lhsT=VS[kt][:, h * 65 : h * 65 + 65],
                                rhs=PT[i % 6][:, rr * 512 + w0 : rr * 512 + 512],
                                start=(kt == 0),
                                stop=(kt == nkt - 1),
                                skip_group_check=True,
                            )
                            if rr == 1:
                                mm.then_inc(sAV, 1)
                        if kt == nkt - 1:
                            w(sRC, gblk + 1)
                            for rr in range(2):
                                nc.tensor.matmul(
                                    P[0:64, pbc(rr) : pbc(rr) + 512],
                                    lhsT=one64_sb[:],
                                    rhs=RCM[0:1, rr * 512 : rr * 512 + 512],
                                    start=True,
                                    stop=True,
                                    skip_group_check=True,
                                ).then_inc(sPB, 1)
                    # out-projection
                    w(dWO, 128)
                    order = [0, 4, 1, 5, 2, 6, 3, 7]
                    for qt in range(NST):
                        gq = r * NST + qt
                        bank = qt % 2
                        qsl = slice(qt * 128, (qt + 1) * 128)
                        w(sOB, gq - 1)
                        for pos, dtk in enumerate(order):
                            w(dAT, 32 * (NHP * r + (dtk % 4) + 1))
                            mm = nc.tensor.matmul(
                                P[:, bank * 512 : bank * 512 + 512],
                                lhsT=ATB[dtk][:, qsl],
                                rhs=WOt[dtk][:],
                                start=(pos == 0),
                                stop=(pos == 7),
                                skip_group_check=True,
                            )
                        mm.then_inc(sOP, 1)

            @blk.scalar
            def _(scalar):
                w = Waiter(scalar)
                for r in range(reps):
                    for i, (hp, qb, kt, nkt, delta) in enumerate(steps):
                        gi = r * NSTEPS + i
                        w0 = max(delta, 0)
                        s = i % 2
                        w(sPS, gi + 1)
                        w(sAV, gi - 5)
                        src = P[:, s * 1024 : (s + 1) * 1024]
                        dst = PT[i % 6][:, :]
                        if w0 == 0:
                            act = nc.scalar.activation(dst, src, AF.Exp, scale=SCALE)
                        else:
                            sv = src.rearrange("p (t c) -> p t c", t=2)[:, :, w0:512]
                            dv = dst.rearrange("p (t c) -> p t c", t=2)[:, :, w0:512]
                            act = nc.scalar.activation(dv, sv, AF.Exp, scale=SCALE)
                        act.then_inc(sEX, 1)

            @blk.vector
            def _(vector):
                w = Waiter(vector)
                for st in range(NST):
                    vv = VS[st][:, :].rearrange("p (h x) -> p h x", x=65)
                    nc.vector.memset(vv[:, :, 64:65], 1.0)
                for r in range(reps):
                    for j, item in enumerate(proj):
                        gj = r * NPJ + j
                        bank = (j % 4) if item[0] != "v" else 4 + (j % 4)
                        pslc = slice(bank * 512, bank * 512 + 512)
                        w(sPJ, gj + 1)
                        if item[0] in ("q", "k"):
                            kind, hp, sb = item
                            if r > 0:
                                w(sAV, (r - 1) * NSTEPS + hp_last[hp] + 1)
                            dst = (QT if kind == "q" else KT)[hp]
                            ssl = slice(sb * 512, (sb + 1) * 512)
                            nc.vector.tensor_copy(dst[:, ssl], P[:, pslc]).then_inc(
                                sPJC, 1
                            )
                        else:
                            _, st = item
                            if r > 0:
                                w(sAV, (r - 1) * NSTEPS + vs_last[st] + 1)
                            vv = VS[st][:, :].rearrange("p (h x) -> p h x", x=65)
                            nc.vector.tensor_copy(
                                vv[:, :, 0:64],
                                P[:, pslc].rearrange("p (h x) -> p h x", x=64),
                            ).then_inc(sPJC, 1)
                    for i, (hp, qb, kt, nkt, delta) in enumerate(steps):
                        gi = r * NSTEPS + i
                        gblk = r * NBLK + hp * NQB + qb
                        w0 = max(delta, 0)
                        if kt == nkt - 1:
                            qsl = slice(qb * 512, (qb + 1) * 512)
                            w(sAV, gi + 1)
                            nc.vector.reciprocal(
                                RCM[:], P[64:65, 2048:3072]
                            ).then_inc(sRC, 1)
                            for rr in range(2):
                                h = 2 * hp + rr
                                w(sPB, 2 * gblk + rr + 1)
                                if r > 0:
                                    w(dCC, 16 * (NHP * (r - 1) + hp + 1))
                                nc.vector.tensor_copy(
                                    PBS[rr][:], P[0:64, pbc(rr) : pbc(rr) + 512]
                                )
                                nc.vector.tensor_mul(
                                    ASBP[hp][rr * 64 : (rr + 1) * 64, qsl],
                                    P[0:64, pav(rr) : pav(rr) + 512],
                                    PBS[rr][:],
                                ).then_inc(sA, 1)
                    for qt in range(NST):
                        gq = r * NST + qt
                        w(sOP, gq + 1)
                        if gq >= 2:
                            w(dO[qt % 2], 16 * (r * (NST // 2) + qt // 2))
                        bank = qt % 2
                        nc.vector.tensor_add(
                            OSB[qt % 2][:],
                            P[:, bank * 512 : bank * 512 + 512],
                            bob_sb[:],
                        ).then_inc(sOB, 1)

    return nc


_cached = {}


def _get_program(S=S_FULL, reps=1):
    key = (S, reps)
    if key not in _cached:
        _cached[key] = build_program(S, reps)
    return _cached[key]


def make_in_maps(x, Wq, Wk, Wv, Wo, bo):
    bf = ml_dtypes.bfloat16
    tri01 = (np.arange(512)[None, :] >= np.arange(128)[:, None]).astype(bf)
    ntri01 = (np.arange(896)[None, :] < (np.arange(128)[:, None] + 384)).astype(bf)
    negi01 = (np.eye(128) * -60000.0).astype(bf)
    ones64 = np.ones((1, 64), np.float32)
    x = np.asarray(x)
    # each batch's transposed activations feed both TP halves: build once
    xtb = [np.ascontiguousarray(x[b].T).astype(bf) for b in range(B)]
    in_maps = []
    for c in range(NCORES):
        b, p = divmod(c, 2)
        dsl = slice(p * DOWN, (p + 1) * DOWN)
        in_maps.append(
            {
                "xt": xtb[b],
                "wq": np.ascontiguousarray(np.asarray(Wq)[:, dsl]).astype(bf),
                "wk": np.ascontiguousarray(np.asarray(Wk)[:, dsl]).astype(bf),
                "wv": np.ascontiguousarray(np.asarray(Wv)[:, dsl]).astype(bf),
                "wo": np.ascontiguousarray(np.asarray(Wo)[:, dsl]).astype(bf),
                "bob": np.tile(np.asarray(bo, np.float32)[dsl], (128, 1)),
                "tri": tri01,
                "ntri": ntri01,
                "negi": negi01,
                "one64": ones64,
            }
        )
    return in_maps


def assemble(results, S):
    out = np.empty((B, S, D), np.float32)
    for c in range(NCORES):
        b, p = divmod(c, 2)
        out[b, :, p * DOWN : (p + 1) * DOWN] = results[c]["out"]
    return out


def kernel(**inputs):
    x = np.asarray(inputs["x"], np.float32)
    S = x.shape[1]
    nc = _get_program(S)
    in_maps = make_in_maps(
        x,
        inputs["Wq"],
        inputs["Wk"],
        inputs["Wv"],
        inputs["Wo"],
        inputs["bo"],
    )
    res = run_bass_kernel_spmd(nc, in_maps, core_ids=list(range(NCORES)))
    return assemble(res.results, S)



# revision 2
# speedup vs baseline: 1.6231x; 1.6231x over previous
"""Trainium2 Bass kernel for nn_MultiHeadCausalAttention (B=4, S=2048, D=1024, H=16).

Sharding: 8 cores = 4 (batch) x 2 (tensor-parallel over heads; 8 heads/core).
Per core:
  - QKV projections for its 8 heads, computed from x^T (host-transposed).
  - Flash-style causal attention in transposed-score layout: S^T = K @ Q^T with
    k on partitions, so exp(S^T) feeds the P^T @ V matmul directly (contraction
    over k) with no on-chip transposes.
  - Softmax denominators come from col-tiled companion matmuls: each AV step
    issues 4 M=64 matmuls — A_h0 (rows 0-63) and A_h1 (rows 64-127) into an
    "A" bank, plus ones-weighted copies d_h0/d_h1 into a "d" bank at the SAME
    partition ranges.  The denominator therefore lands broadcast across the
    partitions of its head, so normalization is one full-width DVE reciprocal
    + one multiply (the single-partition reciprocal + rank-1 broadcast matmul
    of the previous design was a 6.8us PE stall per block that also kept
    re-throttling the PE clock gate).  A/d bank pairs rotate per block so the
    DVE normalization runs concurrently with the next block's matmuls.
  - AllGather (pairs) of the per-head attention outputs A^T so each core can
    apply the full out-projection for its 512 output columns (host-sliced Wo).
Raw Bass (no Tile): per-engine programs with hand-placed counting semaphores.
`reps` replays the body inside one NEFF (sem values offset per rep) so the
true per-iteration time can be measured as a slope, independent of the ~78 ms
axon dispatch floor.
Host: transposes x, slices/casts weights to bf16, assembles the output halves.
"""

from contextlib import ExitStack

import numpy as np
import ml_dtypes

import concourse.bass as bass
import concourse.mybir as mybir
from concourse.bass_utils import run_bass_kernel_spmd

F32 = mybir.dt.float32
BF16 = mybir.dt.bfloat16
AF = mybir.ActivationFunctionType

B, S_FULL, D = 4, 2048, 1024
NCORES = 8
NDT = D // 128
DOWN = D // 2          # output dims owned per core (8 heads * 64)
SCALE = 1.0 / 32.0     # d_out ** -0.5
RG = [[0, 1], [2, 3], [4, 5], [6, 7]]

# How the first matmul of each per-bank accumulation chain marks start= when
# two col-tiled chains share a bank (see microtest_psum.py):
#   "both"  - every chain's first matmul uses start=True (region-scoped clear)
#   "first" - only the first chain into the bank uses start=True (whole-bank
#             clear; the second chain overwrites into cleared bits)
CHAIN_START = "both"


class Waiter:
    """Per-engine wait helper that elides waits already implied."""

    def __init__(self, eng):
        self.eng = eng
        self.seen = {}

    def __call__(self, sem, val):
        if val <= 0:
            return
        if self.seen.get(sem.name, -1) >= val:
            return
        self.seen[sem.name] = val
        self.eng.wait_ge(sem, val)


def build_program(S=S_FULL, reps=1):
    NQB = S // 512
    NST = S // 128
    NHP = 4
    NBLK = NHP * NQB

    # attention step list
    steps = []
    for hp in range(NHP):
        for qb in range(NQB):
            nkt = (qb + 1) * 4
            for kt in range(nkt):
                steps.append((hp, qb, kt, nkt, kt * 128 - qb * 512))
    NSTEPS = len(steps)
    # last attention step index touching head-pair hp / V tile st
    hp_last = [max(i for i, s in enumerate(steps) if s[0] == hp) for hp in range(NHP)]
    vs_last = [
        max(i for i, s in enumerate(steps) if s[2] == st) for st in range(NST)
    ]

    # proj emission order shared by PE and DVE
    proj = []
    for hp in range(NHP):
        for sb in range(S // 512):
            proj.append(("q", hp, sb))
            proj.append(("k", hp, sb))
    for st in range(NST):
        proj.append(("v", st))
    NPJ = len(proj)

    nc = bass.Bass()
    xt = nc.declare_dram_parameter("xt", [D, S], BF16, isOutput=False)
    wq = nc.declare_dram_parameter("wq", [D, DOWN], BF16, isOutput=False)
    wk = nc.declare_dram_parameter("wk", [D, DOWN], BF16, isOutput=False)
    wv = nc.declare_dram_parameter("wv", [D, DOWN], BF16, isOutput=False)
    wo = nc.declare_dram_parameter("wo", [D, DOWN], BF16, isOutput=False)
    bob = nc.declare_dram_parameter("bob", [128, DOWN], F32, isOutput=False)
    ntri = nc.declare_dram_parameter("ntri", [128, 896], BF16, isOutput=False)
    negi = nc.declare_dram_parameter("negi", [128, 128], BF16, isOutput=False)
    out = nc.declare_dram_parameter("out", [S, DOWN], F32, isOutput=True)

    cci = [nc.dram_tensor(f"cci{i}", [128, S], BF16) for i in range(NHP)]
    cco = [nc.dram_tensor(f"cco{i}", [256, S], BF16) for i in range(NHP)]

    with ExitStack() as ctx:
        e = ctx.enter_context
        ctx.enter_context(
            nc.allow_low_precision(reason="intentional bf16 flash attention")
        )

        sems = {}
        for n in (
            "dXQ", "dWK", "dWV", "dWO", "dMISC", "sPJ", "sPJC", "sPS", "sEX",
            "sAV", "sA", "dCC", "sCG", "dAT", "sOP", "sOB", "dO0", "dO1",
            "sON",
        ):
            sems[n] = e(nc.semaphore(n))
        dXQ, dWK, dWV, dWO, dMISC = (sems[k] for k in ("dXQ", "dWK", "dWV", "dWO", "dMISC"))
        sPJ, sPJC, sPS, sEX = (sems[k] for k in ("sPJ", "sPJC", "sPS", "sEX"))
        sAV, sA = (sems[k] for k in ("sAV", "sA"))
        dCC, sCG, dAT, sOP, sOB = (sems[k] for k in ("dCC", "sCG", "dAT", "sOP", "sOB"))
        sON = sems["sON"]
        dO = [sems["dO0"], sems["dO1"]]

        # one PSUM tensor, manual bank layout:
        # banks 0-3: proj psums (q/k) & attention score slots & outproj (0-1)
        # banks 4-7: proj psums (v); during attention: A/d bank pairs,
        #            rotating per block (even blocks 4-5, odd blocks 6-7)
        P = e(nc.psum_tensor("P", [128, 4096], F32))

        QT = [e(nc.sbuf_tensor(f"QT{i}", [128, S], BF16)) for i in range(NHP)]
        KT = [e(nc.sbuf_tensor(f"KT{i}", [128, S], BF16)) for i in range(NHP)]
        VS = [e(nc.sbuf_tensor(f"VS{i}", [128, 512], BF16)) for i in range(NST)]
        WOt = [e(nc.sbuf_tensor(f"WOt{i}", [128, DOWN], BF16)) for i in range(NDT)]
        ntri_sb = e(nc.sbuf_tensor("ntri_sb", [128, 896], BF16))
        negi_sb = e(nc.sbuf_tensor("negi_sb", [128, 128], BF16))
        ones_sb = e(nc.sbuf_tensor("ones_sb", [128, 64], BF16))
        bob_sb = e(nc.sbuf_tensor("bob_sb", [128, DOWN], F32))
        PT = [e(nc.sbuf_tensor(f"PT{i}", [128, 1024], BF16)) for i in range(6)]
        RSB = e(nc.sbuf_tensor("RSB", [128, 512], F32))
        OSB = [e(nc.sbuf_tensor(f"OSB{i}", [128, DOWN], F32)) for i in range(2)]
        XT = [e(nc.sbuf_tensor(f"XT{i}", [128, S], BF16)) for i in range(NDT)]
        WQt = [e(nc.sbuf_tensor(f"WQt{i}", [128, DOWN], BF16)) for i in range(NDT)]
        WKt = [e(nc.sbuf_tensor(f"WKt{i}", [128, DOWN], BF16)) for i in range(NDT)]
        WVt = [e(nc.sbuf_tensor(f"WVt{i}", [128, DOWN], BF16)) for i in range(NDT)]
        ATB = [e(nc.sbuf_tensor(f"ATB{i}", [128, S], BF16)) for i in range(NDT)]
        # A^T head pairs: head 2hp+r lives on partitions r*64:(r+1)*64
        ASBP = [e(nc.sbuf_tensor(f"ASBP{i}", [128, S], BF16)) for i in range(NHP)]

        with nc.Block() as blk:

            @blk.sync
            def _(sync):
                w = Waiter(sync)
                for r in range(reps):
                    if r == 0:
                        for i in range(NDT):
                            sl = slice(i * 128, (i + 1) * 128)
                            sync.dma_start(XT[i][:], xt[sl, :]).then_inc(dXQ, 16)
                            sync.dma_start(WQt[i][:], wq[sl, :]).then_inc(dXQ, 16)
                        for i in range(NDT):
                            sl = slice(i * 128, (i + 1) * 128)
                            sync.dma_start(WKt[i][:], wk[sl, :]).then_inc(dWK, 16)
                        for i in range(NDT):
                            sl = slice(i * 128, (i + 1) * 128)
                            sync.dma_start(WVt[i][:], wv[sl, :]).then_inc(dWV, 16)
                        sync.dma_start(ntri_sb[:], ntri[:]).then_inc(dMISC, 16)
                        sync.dma_start(negi_sb[:], negi[:]).then_inc(dMISC, 16)
                        sync.dma_start(bob_sb[:], bob[:]).then_inc(dMISC, 16)
                        for i in range(NDT):
                            sl = slice(i * 128, (i + 1) * 128)
                            sync.dma_start(WOt[i][:], wo[sl, :]).then_inc(dWO, 16)
                    for hp in range(NHP):
                        w(sA, r * NBLK + hp * NQB + NQB)
                        if r > 0:
                            w(sCG, NHP * (r - 1) + hp + 1)  # cci free
                        sync.dma_start(cci[hp][:], ASBP[hp][:]).then_inc(dCC, 16)
                    if r + 1 < reps:
                        # re-stream x for the next rep (steady-state measurement)
                        w(sPJ, (r + 1) * NPJ)  # this rep's proj done reading XT
                        for i in range(NDT):
                            sl = slice(i * 128, (i + 1) * 128)
                            sync.dma_start(XT[i][:], xt[sl, :]).then_inc(dXQ, 16)
                    for qt in range(NST):
                        gq = r * NST + qt
                        w(sOB, gq + 1)
                        sync.dma_start(
                            out[qt * 128 : (qt + 1) * 128, :], OSB[qt % 2][:]
                        ).then_inc(dO[qt % 2], 16)
                w(dO[0], 16 * reps * ((NST + 1) // 2))
                w(dO[1], 16 * reps * (NST // 2))

            @blk.gpsimd
            def _(gpsimd):
                w = Waiter(gpsimd)
                for r in range(reps):
                    for hp in range(NHP):
                        w(dCC, 16 * (NHP * r + hp + 1))
                        if r > 0:
                            w(dAT, 32 * (NHP * (r - 1) + hp + 1))  # cco free
                        gpsimd.collective_compute(
                            "AllGather",
                            mybir.AluOpType.bypass,
                            replica_groups=RG,
                            ins=[cci[hp][:]],
                            outs=[cco[hp][:]],
                        ).then_inc(sCG, 1)
                        w(sCG, NHP * r + hp + 1)
                        if r > 0:
                            w(sOP, r * NST)  # ATB free (prev outproj done)
                        gpsimd.dma_start(ATB[hp][:], cco[hp][0:128, :]).then_inc(
                            dAT, 16
                        )
                        gpsimd.dma_start(
                            ATB[hp + 4][:], cco[hp][128:256, :]
                        ).then_inc(dAT, 16)

            @blk.tensor
            def _(tensor):
                w = Waiter(tensor)
                for r in range(reps):
                    # psum banks from previous rep fully consumed?
                    if r > 0:
                        w(sEX, r * NSTEPS)
                        w(sA, r * NBLK)
                        w(sOB, r * NST)
                    def dxq_val(dt):
                        # bulk wait: DMA queue completions are unordered, so
                        # only the all-issued count is a sound threshold
                        return 256 + 128 * r

                    for j, item in enumerate(proj):
                        gj = r * NPJ + j
                        bank = (j % 4) if item[0] != "v" else 4 + (j % 4)
                        pslc = slice(bank * 512, bank * 512 + 512)
                        w(sPJC, gj - 3)
                        if item[0] in ("q", "k"):
                            kind, hp, sb = item
                            wt = WQt if kind == "q" else WKt
                            hsl = slice(hp * 128, (hp + 1) * 128)
                            ssl = slice(sb * 512, (sb + 1) * 512)
                            for dt in range(NDT):
                                w(dXQ, dxq_val(dt))
                                if kind == "k":
                                    w(dWK, 128)
                                mm = nc.tensor.matmul(
                                    P[:, pslc],
                                    lhsT=wt[dt][:, hsl],
                                    rhs=XT[dt][:, ssl],
                                    start=(dt == 0),
                                    stop=(dt == NDT - 1),
                                    skip_group_check=True,
                                )
                            mm.then_inc(sPJ, 1)
                        else:
                            _, st = item
                            stsl = slice(st * 128, (st + 1) * 128)
                            for dt in range(NDT):
                                w(dXQ, dxq_val(dt))
                                w(dWV, 128)
                                mm = nc.tensor.matmul(
                                    P[:, pslc],
                                    lhsT=XT[dt][:, stsl],
                                    rhs=WVt[dt][:],
                                    start=(dt == 0),
                                    stop=(dt == NDT - 1),
                                    skip_group_check=True,
                                )
                            mm.then_inc(sPJ, 1)
                    # attention (PE software-pipelined: scores run one
                    # step ahead of the AV matmuls so ACT exp overlaps PE)
                    w(sPJC, (r + 1) * NPJ)
                    w(dMISC, 48)
                    if r == 0:
                        w(sON, 1)

                    def emit_scores(i):
                        hp, qb, kt, nkt, delta = steps[i]
                        gi = r * NSTEPS + i
                        s = i % 2
                        qsl = slice(qb * 512, (qb + 1) * 512)
                        ksl = slice(kt * 128, (kt + 1) * 128)
                        w(sEX, gi - 1)
                        diag = delta >= 0
                        for rr in range(2):
                            psl = slice(rr * 64, (rr + 1) * 64)
                            mm = nc.tensor.matmul(
                                P[:, s * 1024 + rr * 512 : s * 1024 + rr * 512 + 512],
                                lhsT=KT[hp][psl, ksl],
                                rhs=QT[hp][psl, qsl],
                                start=True,
                                stop=not diag,
                                tile_position=(rr * 64, 0),
                                skip_group_check=True,
                            )
                        if diag:
                            # causal mask: accumulate -BIG onto j < delta + p
                            wsl = slice(384, 896 - delta)
                            for rr in range(2):
                                base = s * 1024 + rr * 512
                                mm = nc.tensor.matmul(
                                    P[:, base + delta : base + 512],
                                    lhsT=negi_sb[:],
                                    rhs=ntri_sb[:, wsl],
                                    start=False,
                                    stop=True,
                                    skip_group_check=True,
                                )
                        mm.then_inc(sPS, 1)

                    emit_scores(0)
                    for i, (hp, qb, kt, nkt, delta) in enumerate(steps):
                        gi = r * NSTEPS + i
                        gblk = r * NBLK + hp * NQB + qb
                        blk_idx = hp * NQB + qb
                        w0 = max(delta, 0)
                        if i + 1 < NSTEPS:
                            emit_scores(i + 1)
                        w(sEX, gi + 1)
                        # A/d bank pair for this block (rotates per block)
                        ab = 2048 + (blk_idx % 2) * 1024
                        db = ab + 512
                        # the pair is free once block gblk-2's mul is done
                        w(sA, gblk - 1)
                        first = kt == 0
                        last = kt == nkt - 1
                        ysta = first if CHAIN_START == "both" else False
                        h0 = 2 * hp
                        pt = PT[i % 6]
                        mm = nc.tensor.matmul(
                            P[0:64, ab + w0 : ab + 512],
                            lhsT=VS[kt][:, h0 * 64 : h0 * 64 + 64],
                            rhs=pt[:, w0:512],
                            start=first,
                            stop=last,
                            skip_group_check=True,
                        )
                        mm = nc.tensor.matmul(
                            P[64:128, ab + w0 : ab + 512],
                            lhsT=VS[kt][:, h0 * 64 + 64 : h0 * 64 + 128],
                            rhs=pt[:, 512 + w0 : 1024],
                            start=ysta,
                            stop=last,
                            skip_group_check=True,
                        )
                        mm = nc.tensor.matmul(
                            P[0:64, db + w0 : db + 512],
                            lhsT=ones_sb[:],
                            rhs=pt[:, w0:512],
                            start=first,
                            stop=last,
                            skip_group_check=True,
                        )
                        mm = nc.tensor.matmul(
                            P[64:128, db + w0 : db + 512],
                            lhsT=ones_sb[:],
                            rhs=pt[:, 512 + w0 : 1024],
                            start=ysta,
                            stop=last,
                            skip_group_check=True,
                        )
                        mm.then_inc(sAV, 1)
                    # out-projection
                    w(dWO, 128)
                    order = [0, 4, 1, 5, 2, 6, 3, 7]
                    for qt in range(NST):
                        gq = r * NST + qt
                        bank = qt % 2
                        qsl = slice(qt * 128, (qt + 1) * 128)
                        w(sOB, gq - 1)
                        for pos, dtk in enumerate(order):
                            w(dAT, 32 * (NHP * r + (dtk % 4) + 1))
                            mm = nc.tensor.matmul(
                                P[:, bank * 512 : bank * 512 + 512],
                                lhsT=ATB[dtk][:, qsl],
                                rhs=WOt[dtk][:],
                                start=(pos == 0),
                                stop=(pos == 7),
                                skip_group_check=True,
                            )
                        mm.then_inc(sOP, 1)

            @blk.scalar
            def _(scalar):
                w = Waiter(scalar)
                for r in range(reps):
                    for i, (hp, qb, kt, nkt, delta) in enumerate(steps):
                        gi = r * NSTEPS + i
                        w0 = max(delta, 0)
                        s = i % 2
                        w(sPS, gi + 1)
                        w(sAV, gi - 5)
                        src = P[:, s * 1024 : (s + 1) * 1024]
                        dst = PT[i % 6][:, :]
                        if w0 == 0:
                            act = nc.scalar.activation(dst, src, AF.Exp, scale=SCALE)
                        else:
                            sv = src.rearrange("p (t c) -> p t c", t=2)[:, :, w0:512]
                            dv = dst.rearrange("p (t c) -> p t c", t=2)[:, :, w0:512]
                            act = nc.scalar.activation(dv, sv, AF.Exp, scale=SCALE)
                        act.then_inc(sEX, 1)

            @blk.vector
            def _(vector):
                w = Waiter(vector)
                nc.vector.memset(ones_sb[:], 1.0).then_inc(sON, 1)
                for r in range(reps):
                    for j, item in enumerate(proj):
                        gj = r * NPJ + j
                        bank = (j % 4) if item[0] != "v" else 4 + (j % 4)
                        pslc = slice(bank * 512, bank * 512 + 512)
                        w(sPJ, gj + 1)
                        if item[0] in ("q", "k"):
                            kind, hp, sb = item
                            if r > 0:
                                w(sAV, (r - 1) * NSTEPS + hp_last[hp] + 1)
                            dst = (QT if kind == "q" else KT)[hp]
                            ssl = slice(sb * 512, (sb + 1) * 512)
                            nc.vector.tensor_copy(dst[:, ssl], P[:, pslc]).then_inc(
                                sPJC, 1
                            )
                        else:
                            _, st = item
                            if r > 0:
                                w(sAV, (r - 1) * NSTEPS + vs_last[st] + 1)
                            nc.vector.tensor_copy(
                                VS[st][:, :], P[:, pslc]
                            ).then_inc(sPJC, 1)
                    for i, (hp, qb, kt, nkt, delta) in enumerate(steps):
                        gi = r * NSTEPS + i
                        gblk = r * NBLK + hp * NQB + qb
                        blk_idx = hp * NQB + qb
                        if kt == nkt - 1:
                            qsl = slice(qb * 512, (qb + 1) * 512)
                            ab = 2048 + (blk_idx % 2) * 1024
                            db = ab + 512
                            w(sAV, gi + 1)
                            if r > 0:
                                w(dCC, 16 * (NHP * (r - 1) + hp + 1))
                            nc.vector.reciprocal(RSB[:], P[:, db : db + 512])
                            nc.vector.tensor_mul(
                                ASBP[hp][:, qsl],
                                P[:, ab : ab + 512],
                                RSB[:],
                            ).then_inc(sA, 1)
                    for qt in range(NST):
                        gq = r * NST + qt
                        w(sOP, gq + 1)
                        if gq >= 2:
                            w(dO[qt % 2], 16 * (r * (NST // 2) + qt // 2))
                        bank = qt % 2
                        nc.vector.tensor_add(
                            OSB[qt % 2][:],
                            P[:, bank * 512 : bank * 512 + 512],
                            bob_sb[:],
                        ).then_inc(sOB, 1)

    return nc


_cached = {}


def _get_program(S=S_FULL, reps=1):
    key = (S, reps)
    if key not in _cached:
        _cached[key] = build_program(S, reps)
    return _cached[key]


def make_in_maps(x, Wq, Wk, Wv, Wo, bo):
    bf = ml_dtypes.bfloat16
    ntri01 = (np.arange(896)[None, :] < (np.arange(128)[:, None] + 384)).astype(bf)
    negi01 = (np.eye(128) * -60000.0).astype(bf)
    x = np.asarray(x)
    # each batch's transposed activations feed both TP halves: build once
    xtb = [np.ascontiguousarray(x[b].T).astype(bf) for b in range(B)]
    in_maps = []
    for c in range(NCORES):
        b, p = divmod(c, 2)
        dsl = slice(p * DOWN, (p + 1) * DOWN)
        in_maps.append(
            {
                "xt": xtb[b],
                "wq": np.ascontiguousarray(np.asarray(Wq)[:, dsl]).astype(bf),
                "wk": np.ascontiguousarray(np.asarray(Wk)[:, dsl]).astype(bf),
                "wv": np.ascontiguousarray(np.asarray(Wv)[:, dsl]).astype(bf),
                "wo": np.ascontiguousarray(np.asarray(Wo)[:, dsl]).astype(bf),
                "bob": np.tile(np.asarray(bo, np.float32)[dsl], (128, 1)),
                "ntri": ntri01,
                "negi": negi01,
            }
        )
    return in_maps


def assemble(results, S):
    out = np.empty((B, S, D), np.float32)
    for c in range(NCORES):
        b, p = divmod(c, 2)
        out[b, :, p * DOWN : (p + 1) * DOWN] = results[c]["out"]
    return out


def kernel(**inputs):
    x = np.asarray(inputs["x"], np.float32)
    S = x.shape[1]
    nc = _get_program(S)
    in_maps = make_in_maps(
        x,
        inputs["Wq"],
        inputs["Wk"],
        inputs["Wv"],
        inputs["Wo"],
        inputs["bo"],
    )
    res = run_bass_kernel_spmd(nc, in_maps, core_ids=list(range(NCORES)))
    return assemble(res.results, S)
